# revision 8
# baseline (speedup 1.0000x reference)
"""GRU Seq2Seq Trainium2 kernel (nn_GRU_Seq2Seq_83219286327778).

Strategy: data-parallel over batch (2048 -> 8 x 256), gate-major transposed
layout on-device ([hidden/gate dim on partitions, batch on free dim]) so the
recurrence needs no transposes.

v2: fp16 matmul operands (weights, hidden state, src) with fp32 PSUM
accumulation; biases applied via the ACT engine's per-partition bias operand
(sigmoid) and the DVE scalar_tensor_tensor scalar slots (tanh path) instead
of rank-1 matmuls; per-cell matmuls ordered gh-first so the recurrent-side
matmuls (which depend only on state from two cells back) fill the tensor
engine while the previous cell's elementwise tail completes; h-update runs
in fp16 SBUF (DVE 4x mode); fc4 feedback folded into the next step's gx via
Wcomb = dW0 @ W4; all weights resident in SBUF from the start.
"""
import sys
sys.path.insert(0, "/opt/trn_rl_repo")
import numpy as np

F16NP = np.float16

B, LAGS, HORIZONS, F, H = 2048, 64, 24, 64, 512
NCORES = 8
BL = B // NCORES           # 256 batch per core
G3 = 3 * H                 # 1536
KC = H // 128              # 4 k-chunks
SRC_CHUNK = 8              # timesteps per src DMA

# btab column layout (bias table, [128, 52] fp32):
#   cn   (stt-t scalar):  cols  0..15  = ctype {el0,el1,dl0,dl1} * 4 + m
#   rz   (ACT bias):      cols 16..39  = 16 + ctype {el1,dl0r,dl1}*8 + gate*4 + m
#   bn   (stt-v scalar):  cols 40..51  = 40 + ctype {el1,dl0r,dl1}*4 + m
CN_EL0, CN_EL1, CN_DL0, CN_DL1 = 0, 1, 2, 3
RZ_EL1, RZ_DL0R, RZ_DL1 = 0, 1, 2
NBCOL = 52

_RUNNER = None

# matmul operand dtype ('float16' | 'float32r') and fp16-elementwise toggle
MM_DT = "float16"
EW16 = True


def _build_nc(repeat=1, lags=LAGS, horizons=HORIZONS, mm_dt=None, ew16=None):
    import concourse.tile as tile
    from concourse import mybir, bacc

    mm_dt = MM_DT if mm_dt is None else mm_dt
    ew16 = EW16 if ew16 is None else ew16
    F32 = mybir.dt.float32
    F16 = getattr(mybir.dt, mm_dt)
    E16 = mybir.dt.float16 if ew16 else F32
    # DRAM dtype must match the numpy arrays the runner feeds (float32r
    # tiles are bit-identical to fp32, so DMA from an F32 dram tensor)
    FD = F32 if mm_dt == "float32r" else F16
    AF = mybir.ActivationFunctionType
    OP = mybir.AluOpType

    nc = bacc.Bacc("TRN2", target_bir_lowering=False)

    srcT_d = nc.dram_tensor("srcT", [F + 1, LAGS, BL], FD, kind="ExternalInput")
    wnames = ["eu0", "ew1", "eu1", "du0", "dw1", "du1", "wcomb"]
    w_d = {n: nc.dram_tensor(n, [H, G3], FD, kind="ExternalInput") for n in wnames}
    ew0a_d = nc.dram_tensor("ew0a", [F + 1, G3], FD, kind="ExternalInput")
    dw0a_d = nc.dram_tensor("dw0a", [F + 1, G3], FD, kind="ExternalInput")
    btab_d = nc.dram_tensor("btab", [128, NBCOL], F32, kind="ExternalInput")
    w1t_d = nc.dram_tensor("w1t", [128, KC], FD, kind="ExternalInput")
    out_d = nc.dram_tensor("out", [HORIZONS, BL], F32, kind="ExternalOutput")

    with tile.TileContext(nc) as tc:
        with tc.tile_pool(name="wp", bufs=1) as wp, \
             tc.tile_pool(name="sp", bufs=2) as sp, \
             tc.tile_pool(name="hp", bufs=1) as hp, \
             tc.tile_pool(name="gp", bufs=2) as gp, \
             tc.tile_pool(name="op_", bufs=2) as opool, \
             tc.tile_pool(name="pp", bufs=1, space="PSUM") as pp:

            # ---- persistent small tensors ----
            btab_t = wp.tile([128, NBCOL], F32, tag="btab", name="btab")
            nc.sync.dma_start(btab_t[:], btab_d[:])
            w1t_t = wp.tile([128, KC], F16, tag="w1t", name="w1t")
            nc.gpsimd.dma_start(w1t_t[:], w1t_d[:])
            ew0a_t = wp.tile([F + 1, G3], F16, tag="w0a", name="w0a")
            nc.gpsimd.dma_start(ew0a_t[:], ew0a_d[:])
            dw0a_t = wp.tile([F + 1, G3], F16, tag="dw0a", name="dw0a")
            nc.gpsimd.dma_start(dw0a_t[:], dw0a_d[:])

            _dmae = [nc.gpsimd, nc.sync]

            def load_u(name):
                ts_ = []
                for k in range(KC):
                    t = wp.tile([128, G3], F16, tag=f"{name}{k}", name=f"{name}{k}")
                    _dmae[k % 2].dma_start(t[:], w_d[name][k * 128:(k + 1) * 128, :])
                    ts_.append(t)
                return ts_

            wt = {n: load_u(n) for n in wnames}

            # hidden state ping-pong, fp16 (matmul rhs + h-update operand)
            h0b = [hp.tile([128, KC, BL], F16, tag=f"h0{i}", name=f"h0{i}") for i in range(2)]
            h1b = [hp.tile([128, KC, BL], F16, tag=f"h1{i}", name=f"h1{i}") for i in range(2)]

            def cell(gx_rhs, gx_lhs, gh_lhs, h_prev, h_out, cn_ct, rz_ct, bn_ct):
                """One GRU cell, gate-major layout, gh-first matmul order.

                gx_rhs/gx_lhs: k-chunk lists for the input projection
                gh_lhs: KC lhsT tiles for the recurrent projection
                cn_ct: ctype index into btab cn columns (always set)
                rz_ct: ctype index into btab rz columns, or None (biases folded
                       into gx via the ones-row of the [F+1] input)
                bn_ct: ctype index into btab bn columns, or None (ones-row)
                """
                pa = [pp.tile([128, 512], F32, tag=f"pA{m}", name=f"pA{m}") for m in range(4)]
                pb = [pp.tile([128, 512], F32, tag=f"pB{m}", name=f"pB{m}") for m in range(4)]
                rz = gp.tile([128, KC, 2, BL], E16, tag="rz", name="rz")
                tt = gp.tile([128, KC, BL], F32, tag="tt", name="tt")
                vv = gp.tile([128, KC, BL], F32, tag="vv", name="vv")
                nn = gp.tile([128, KC, BL], E16, tag="nn", name="nn")
                uu = gp.tile([128, KC, BL], E16, tag="uu", name="uu")
                pw = gp.tile([128, KC, BL], E16, tag="pw", name="pw")

                ngh = len(gh_lhs)
                ngx = len(gx_lhs)
                # DVE reads of float32r tiles need an fp32 view
                h_ew = h_prev if mm_dt != "float32r" else h_prev.bitcast(F32)

                def mm_gh(out_ap, g, bank_first=False, close=False):
                    # start=True ONLY on the first matmul into the bank this
                    # cell: it clears has_written for the WHOLE bank. Later
                    # regions initialize via the per-element overwrite-on-
                    # unset-bit behavior (flags=0).
                    for k in range(ngh):
                        nc.tensor.matmul(out_ap, gh_lhs[k][:, g * 128:(g + 1) * 128],
                                         h_prev[:, k, :],
                                         start=(bank_first and k == 0),
                                         stop=(close and k == ngh - 1),
                                         skip_group_check=True)

                def mm_gx(out_ap, g):
                    for i, (lhs, rhs) in enumerate(zip(gx_lhs, gx_rhs, strict=True)):
                        nc.tensor.matmul(out_ap, lhs[:, g * 128:(g + 1) * 128], rhs,
                                         start=False, stop=(i == ngx - 1),
                                         skip_group_check=True)

                # --- phase A: gh matmuls (depend only on 2-cells-back state)
                for m in range(4):
                    mm_gh(pa[m][:, 0:BL], m, bank_first=True)    # r
                for m in range(4):
                    mm_gh(pa[m][:, BL:2 * BL], 4 + m)            # z
                for m in range(4):
                    mm_gh(pb[m][:, BL:2 * BL], 8 + m, bank_first=True,
                          close=True)                            # hn (gh-only)

                # --- phase B: gx matmuls + sigmoids
                for m in range(4):
                    mm_gx(pa[m][:, 0:BL], m)                     # r close
                    rb = (btab_t[:, 16 + rz_ct * 8 + m:16 + rz_ct * 8 + m + 1]
                          if rz_ct is not None else 0.0)
                    nc.scalar.activation(rz[:, m, 0, :], pa[m][:, 0:BL], AF.Sigmoid,
                                         bias=rb)
                for m in range(4):
                    mm_gx(pb[m][:, 0:BL], 8 + m)                 # xn
                for m in range(4):
                    mm_gx(pa[m][:, BL:2 * BL], 4 + m)            # z close
                    zb = (btab_t[:, 16 + rz_ct * 8 + 4 + m:16 + rz_ct * 8 + 4 + m + 1]
                          if rz_ct is not None else 0.0)
                    nc.scalar.activation(rz[:, m, 1, :], pa[m][:, BL:2 * BL], AF.Sigmoid,
                                         bias=zb)

                # --- phase C: elementwise tail (per-chunk pipelined)
                for m in range(4):
                    # t = (hn + cn) * r
                    nc.vector.scalar_tensor_tensor(
                        tt[:, m, :], pb[m][:, BL:2 * BL],
                        btab_t[:, cn_ct * 4 + m:cn_ct * 4 + m + 1],
                        rz[:, m, 0, :], OP.add, OP.mult)
                    # v = (t + bn) + xn
                    bns = (btab_t[:, 40 + bn_ct * 4 + m:40 + bn_ct * 4 + m + 1]
                           if bn_ct is not None else 0.0)
                    nc.vector.scalar_tensor_tensor(
                        vv[:, m, :], tt[:, m, :], bns, pb[m][:, 0:BL],
                        OP.add, OP.add)
                for mm2 in range(2):
                    s = slice(2 * mm2, 2 * mm2 + 2)
                    nc.scalar.activation(nn[:, s, :], vv[:, s, :], AF.Tanh)
                for mm2 in range(2):
                    s = slice(2 * mm2, 2 * mm2 + 2)
                    # h' = n + z*(h - n)   (all-fp16 SBUF: DVE fast mode)
                    nc.vector.tensor_tensor(uu[:, s, :], h_ew[:, s, :], nn[:, s, :],
                                            OP.subtract)
                    nc.vector.tensor_tensor(pw[:, s, :], uu[:, s, :], rz[:, s, 1, :],
                                            OP.mult)
                    nc.vector.tensor_tensor(h_out[:, s, :], pw[:, s, :], nn[:, s, :],
                                            OP.add)

            for _rep in range(repeat):
                for i in range(2):
                    nc.vector.memzero(h0b[i][:])
                    nc.vector.memzero(h1b[i][:])

                # ---------------- encoder ----------------
                sc = None
                for t in range(lags):
                    if t % SRC_CHUNK == 0:
                        sc = sp.tile([F + 1, SRC_CHUNK, BL], F16, tag="src",
                                     name=f"src{t}")
                        nc.gpsimd.dma_start(sc[:], srcT_d[:, t:t + SRC_CHUNK, :])
                    j = t % SRC_CHUNK
                    p, q = t % 2, (t + 1) % 2
                    cell([sc[:, j, :]], [ew0a_t], wt["eu0"],
                         h0b[p], h0b[q], CN_EL0, None, None)
                    cell([h0b[q][:, k, :] for k in range(KC)], wt["ew1"], wt["eu1"],
                         h1b[p], h1b[q], CN_EL1, RZ_EL1, RZ_EL1)
                sc_last = sc

                # ---------------- decoder ----------------
                for d in range(horizons):
                    p, q = (lags + d) % 2, (lags + d + 1) % 2
                    if d == 0:
                        cell([sc_last[:, (lags - 1) % SRC_CHUNK, :]], [dw0a_t],
                             wt["du0"], h0b[p], h0b[q], CN_DL0, None, None)
                    else:
                        cell([h1b[p][:, k, :] for k in range(KC)], wt["wcomb"],
                             wt["du0"], h0b[p], h0b[q], CN_DL0, RZ_DL0R, RZ_DL0R)
                    cell([h0b[q][:, k, :] for k in range(KC)], wt["dw1"], wt["du1"],
                         h1b[p], h1b[q], CN_DL1, RZ_DL1, RZ_DL1)
                    # out1[d] = W1 . h1_new   (b1 added on host)
                    po = pp.tile([128, 512], F32, tag="pA0", name=f"po{d}")
                    for k in range(KC):
                        nc.tensor.matmul(po[0:1, 0:BL], w1t_t[:, k:k + 1],
                                         h1b[q][:, k, :], start=(k == 0),
                                         stop=(k == KC - 1))
                    osb = opool.tile([1, BL], F32, tag="o1", name=f"o{d}")
                    nc.scalar.copy(osb[:], po[0:1, 0:BL])
                    nc.sync.dma_start(out_d[d:d + 1, :], osb[:])

    nc.compile()
    return nc


def _host_prep(inputs):
    f32 = np.float32
    g = {k: np.asarray(v, dtype=f32) if np.asarray(v).dtype != np.int64 else v
         for k, v in inputs.items()}
    src = np.asarray(inputs["src"], f32)
    eW0, eU0, eb0, ec0 = g["eW0"], g["eU0"], g["eb0"], g["ec0"]
    eW1, eU1, eb1, ec1 = g["eW1"], g["eU1"], g["eb1"], g["ec1"]
    dW0, dU0, db0, dc0 = g["dW0"], g["dU0"], g["db0"], g["dc0"]
    dW1, dU1, db1, dc1 = g["dW1"], g["dU1"], g["db1"], g["dc1"]
    W1, b1, W4, b4 = g["W1"], g["b1"], g["W4"], g["b4"]

    def rzn_bias(b, c):
        # ones-row payload: r/z rows carry b+c, n row carries b only
        return np.concatenate([b[0:H] + c[0:H], b[H:2 * H] + c[H:2 * H], b[2 * H:]])

    Wcomb = (dW0 @ W4).astype(f32)                       # [1536, 512]
    dcomb = (db0 + dW0 @ b4).astype(f32)                 # [1536]

    npd = F16NP if MM_DT == "float16" else np.float32
    bf = lambda a: np.ascontiguousarray(a).astype(npd)
    shared = {
        "eu0": bf(eU0.T), "ew1": bf(eW1.T), "eu1": bf(eU1.T),
        "du0": bf(dU0.T), "dw1": bf(dW1.T), "du1": bf(dU1.T),
        "wcomb": bf(Wcomb.T),
        "ew0a": bf(np.concatenate([eW0.T, rzn_bias(eb0, ec0)[None, :]], 0)),
        "dw0a": bf(np.concatenate([dW0.T, rzn_bias(db0, dc0)[None, :]], 0)),
        "w1t": bf(W1[0].reshape(KC, 128).T),
    }

    btab = np.zeros((128, NBCOL), f32)
    # cn columns (h-side n bias, used in stt-t)
    for ci, c in enumerate((ec0, ec1, dc0, dc1)):
        cn = c[2 * H:]
        for m in range(KC):
            btab[:, ci * 4 + m] = cn[m * 128:(m + 1) * 128]
    # rz bias columns (b+c summed) and bn columns (x-side n bias)
    for ci, (b, c) in enumerate(((eb1, ec1), (dcomb, dc0), (db1, dc1))):
        bc = b + c
        for gate in range(2):
            for m in range(KC):
                btab[:, 16 + ci * 8 + gate * 4 + m] = \
                    bc[gate * H + m * 128:gate * H + (m + 1) * 128]
        bn = b[2 * H:]
        for m in range(KC):
            btab[:, 40 + ci * 4 + m] = bn[m * 128:(m + 1) * 128]
    shared["btab"] = btab

    in_maps = []
    for c in range(NCORES):
        s = src[c * BL:(c + 1) * BL]                     # [256, 64, 64]
        sT = np.ascontiguousarray(s.transpose(2, 1, 0))  # [64, 64, 256]
        sA = np.concatenate([sT, np.ones((1, LAGS, BL), f32)], 0)
        m = dict(shared)
        m["srcT"] = bf(sA)
        in_maps.append(m)
    return in_maps, float(b1[0])


class _Runner:
    """Build-once sharded PJRT runner (axon: 8 NeuronCores)."""

    def __init__(self, nc):
        import jax
        from jax.sharding import Mesh, PartitionSpec
        from jax.experimental.shard_map import shard_map
        from concourse import mybir
        from concourse.bass2jax import (_bass_exec_p, partition_id_tensor,
                                        install_neuronx_cc_hook)
        install_neuronx_cc_hook()
        self.jax = jax
        partition_name = nc.partition_id_tensor.name if nc.partition_id_tensor else None
        in_names, out_names, out_avals, zero_outs = [], [], [], []
        for alloc in nc.m.functions[0].allocations:
            if not isinstance(alloc, mybir.MemoryLocationSet):
                continue
            name = alloc.memorylocations[0].name
            if alloc.kind == "ExternalInput":
                if name != partition_name:
                    in_names.append(name)
            elif alloc.kind == "ExternalOutput":
                out_names.append(name)
                shape = tuple(alloc.tensor_shape)
                dtype = mybir.dt.np(alloc.dtype)
                out_avals.append(jax.core.ShapedArray(shape, dtype))
                zero_outs.append(np.zeros(shape, dtype))
        n_params = len(in_names)
        all_in = list(in_names) + list(out_names)
        if partition_name is not None:
            all_in.append(partition_name)
        self.in_names, self.out_names = in_names, out_names
        self.out_avals, self.zero_outs = out_avals, zero_outs

        def _body(*args):
            operands = list(args)
            if partition_name is not None:
                operands.append(partition_id_tensor())
            return tuple(_bass_exec_p.bind(
                *operands, out_avals=tuple(out_avals), in_names=tuple(all_in),
                out_names=tuple(out_names), lowering_input_output_aliases=(),
                sim_require_finite=True, sim_require_nnan=True, nc=nc))

        devices = jax.devices()[:NCORES]
        self.mesh = Mesh(np.asarray(devices), ("core",))
        in_specs = (PartitionSpec("core"),) * (n_params + len(out_names))
        out_specs = (PartitionSpec("core"),) * len(out_names)
        donate = tuple(range(n_params, n_params + len(out_names)))
        self.fn = jax.jit(
            shard_map(_body, mesh=self.mesh, in_specs=in_specs,
                      out_specs=out_specs, check_rep=False),
            donate_argnums=donate, keep_unused=True)
        self.sh = jax.sharding.NamedSharding(self.mesh, PartitionSpec("core"))

    def place(self, in_maps):
        n = NCORES
        self.placed = [
            self.jax.device_put(np.ascontiguousarray(
                np.concatenate([in_maps[c][nm] for c in range(n)], 0)), self.sh)
            for nm in self.in_names]

    def run(self):
        zeros = [self.jax.device_put(
            np.zeros((NCORES * z.shape[0], *z.shape[1:]), z.dtype), self.sh)
            for z in self.zero_outs]
        outs = self.fn(*self.placed, *zeros)
        self.jax.block_until_ready(outs)
        return outs

    def results(self, outs):
        return [
            {nm: np.asarray(outs[i]).reshape(NCORES, *self.out_avals[i].shape)[c]
             for i, nm in enumerate(self.out_names)}
            for c in range(NCORES)]


def get_runner(repeat=1):
    global _RUNNER
    key = ("r", repeat, MM_DT, EW16)
    if _RUNNER is None or _RUNNER[0] != key:
        nc = _build_nc(repeat=repeat)
        _RUNNER = (key, _Runner(nc))
    return _RUNNER[1]


def kernel(**inputs) -> np.ndarray:
    in_maps, b1 = _host_prep(inputs)
    r = get_runner()
    r.place(in_maps)
    res = r.results(r.run())
    out = np.empty((B, HORIZONS), np.float32)
    for c in range(NCORES):
        out[c * BL:(c + 1) * BL] = res[c]["out"].T + b1
    return out


# revision 10
# speedup vs baseline: 1.0115x; 1.0115x over previous
"""GRU Seq2Seq Trainium2 kernel (nn_GRU_Seq2Seq_83219286327778).

Strategy: data-parallel over batch (2048 -> 8 x 256), gate-major transposed
layout on-device ([hidden/gate dim on partitions, batch on free dim]) so the
recurrence needs no transposes.

v2: fp16 matmul operands (weights, hidden state, src) with fp32 PSUM
accumulation; biases applied via the ACT engine's per-partition bias operand
(sigmoid) and the DVE scalar_tensor_tensor scalar slots (tanh path) instead
of rank-1 matmuls; per-cell matmuls ordered gh-first so the recurrent-side
matmuls (which depend only on state from two cells back) fill the tensor
engine while the previous cell's elementwise tail completes; h-update runs
in fp16 SBUF (DVE 4x mode); fc4 feedback folded into the next step's gx via
Wcomb = dW0 @ W4; all weights resident in SBUF from the start.
"""
import sys
sys.path.insert(0, "/opt/trn_rl_repo")
import numpy as np

F16NP = np.float16

B, LAGS, HORIZONS, F, H = 2048, 64, 24, 64, 512
NCORES = 8
BL = B // NCORES           # 256 batch per core
G3 = 3 * H                 # 1536
KC = H // 128              # 4 k-chunks
SRC_CHUNK = 8              # timesteps per src DMA

# btab column layout (bias table, [128, 52] fp32):
#   cn   (stt-t scalar):  cols  0..15  = ctype {el0,el1,dl0,dl1} * 4 + m
#   rz   (ACT bias):      cols 16..39  = 16 + ctype {el1,dl0r,dl1}*8 + gate*4 + m
#   bn   (stt-v scalar):  cols 40..51  = 40 + ctype {el1,dl0r,dl1}*4 + m
CN_EL0, CN_EL1, CN_DL0, CN_DL1 = 0, 1, 2, 3
RZ_EL1, RZ_DL0R, RZ_DL1 = 0, 1, 2
NBCOL = 52

_RUNNER = None

# matmul operand dtype ('float16' | 'float32r') and fp16-elementwise toggle
MM_DT = "float16"
EW16 = True


def _build_nc(repeat=1, lags=LAGS, horizons=HORIZONS, mm_dt=None, ew16=None):
    import concourse.tile as tile
    from concourse import mybir, bacc

    mm_dt = MM_DT if mm_dt is None else mm_dt
    ew16 = EW16 if ew16 is None else ew16
    F32 = mybir.dt.float32
    F16 = getattr(mybir.dt, mm_dt)
    E16 = mybir.dt.float16 if ew16 else F32
    # DRAM dtype must match the numpy arrays the runner feeds (float32r
    # tiles are bit-identical to fp32, so DMA from an F32 dram tensor)
    FD = F32 if mm_dt == "float32r" else F16
    AF = mybir.ActivationFunctionType
    OP = mybir.AluOpType

    nc = bacc.Bacc("TRN2", target_bir_lowering=False)

    srcT_d = nc.dram_tensor("srcT", [F + 1, LAGS, BL], FD, kind="ExternalInput")
    wnames = ["eu0", "ew1", "eu1", "du0", "dw1", "du1", "wcomb"]
    w_d = {n: nc.dram_tensor(n, [H, G3], FD, kind="ExternalInput") for n in wnames}
    ew0a_d = nc.dram_tensor("ew0a", [F + 1, G3], FD, kind="ExternalInput")
    dw0a_d = nc.dram_tensor("dw0a", [F + 1, G3], FD, kind="ExternalInput")
    btab_d = nc.dram_tensor("btab", [128, NBCOL], F32, kind="ExternalInput")
    w1t_d = nc.dram_tensor("w1t", [128, KC], FD, kind="ExternalInput")
    out_d = nc.dram_tensor("out", [HORIZONS, BL], F32, kind="ExternalOutput")

    with tile.TileContext(nc) as tc:
        with tc.tile_pool(name="wp", bufs=1) as wp, \
             tc.tile_pool(name="sp", bufs=2) as sp, \
             tc.tile_pool(name="hp", bufs=1) as hp, \
             tc.tile_pool(name="gp", bufs=2) as gp, \
             tc.tile_pool(name="op_", bufs=2) as opool, \
             tc.tile_pool(name="pp", bufs=1, space="PSUM") as pp:

            # ---- persistent small tensors ----
            btab_t = wp.tile([128, NBCOL], F32, tag="btab", name="btab")
            nc.sync.dma_start(btab_t[:], btab_d[:])
            w1t_t = wp.tile([128, KC], F16, tag="w1t", name="w1t")
            nc.gpsimd.dma_start(w1t_t[:], w1t_d[:])
            ew0a_t = wp.tile([F + 1, G3], F16, tag="w0a", name="w0a")
            nc.gpsimd.dma_start(ew0a_t[:], ew0a_d[:])
            dw0a_t = wp.tile([F + 1, G3], F16, tag="dw0a", name="dw0a")
            nc.gpsimd.dma_start(dw0a_t[:], dw0a_d[:])

            _dmae = [nc.gpsimd, nc.sync]

            def load_u(name):
                ts_ = []
                for k in range(KC):
                    t = wp.tile([128, G3], F16, tag=f"{name}{k}", name=f"{name}{k}")
                    _dmae[k % 2].dma_start(t[:], w_d[name][k * 128:(k + 1) * 128, :])
                    ts_.append(t)
                return ts_

            wt = {n: load_u(n) for n in wnames}

            # hidden state ping-pong, fp16 (matmul rhs + h-update operand)
            h0b = [hp.tile([128, KC, BL], F16, tag=f"h0{i}", name=f"h0{i}") for i in range(2)]
            h1b = [hp.tile([128, KC, BL], F16, tag=f"h1{i}", name=f"h1{i}") for i in range(2)]

            def cell(gx_rhs, gx_lhs, gh_lhs, h_prev, h_out, cn_ct, rz_ct, bn_ct):
                """One GRU cell, gate-major layout, gh-first matmul order.

                gx_rhs/gx_lhs: k-chunk lists for the input projection
                gh_lhs: KC lhsT tiles for the recurrent projection
                cn_ct: ctype index into btab cn columns (always set)
                rz_ct: ctype index into btab rz columns, or None (biases folded
                       into gx via the ones-row of the [F+1] input)
                bn_ct: ctype index into btab bn columns, or None (ones-row)
                """
                pa = [pp.tile([128, 512], F32, tag=f"pA{m}", name=f"pA{m}") for m in range(4)]
                pb = [pp.tile([128, 512], F32, tag=f"pB{m}", name=f"pB{m}") for m in range(4)]
                rz = gp.tile([128, KC, 2, BL], E16, tag="rz", name="rz")
                tt = gp.tile([128, KC, BL], F32, tag="tt", name="tt")
                vv = gp.tile([128, KC, BL], F32, tag="vv", name="vv")
                nn = gp.tile([128, KC, BL], E16, tag="nn", name="nn")
                uu = gp.tile([128, KC, BL], E16, tag="uu", name="uu")
                pw = gp.tile([128, KC, BL], E16, tag="pw", name="pw")

                ngh = len(gh_lhs)
                ngx = len(gx_lhs)
                # DVE reads of float32r tiles need an fp32 view
                h_ew = h_prev if mm_dt != "float32r" else h_prev.bitcast(F32)

                def mm_gh(out_ap, g, bank_first=False, close=False):
                    # start=True ONLY on the first matmul into the bank this
                    # cell: it clears has_written for the WHOLE bank. Later
                    # regions initialize via the per-element overwrite-on-
                    # unset-bit behavior (flags=0).
                    for k in range(ngh):
                        nc.tensor.matmul(out_ap, gh_lhs[k][:, g * 128:(g + 1) * 128],
                                         h_prev[:, k, :],
                                         start=(bank_first and k == 0),
                                         stop=(close and k == ngh - 1),
                                         skip_group_check=True)

                def mm_gx(out_ap, g):
                    for i, (lhs, rhs) in enumerate(zip(gx_lhs, gx_rhs, strict=True)):
                        nc.tensor.matmul(out_ap, lhs[:, g * 128:(g + 1) * 128], rhs,
                                         start=False, stop=(i == ngx - 1),
                                         skip_group_check=True)

                # --- phase A: gh matmuls (depend only on 2-cells-back state)
                for m in range(4):
                    mm_gh(pa[m][:, 0:BL], m, bank_first=True)    # r
                for m in range(4):
                    mm_gh(pa[m][:, BL:2 * BL], 4 + m)            # z
                for m in range(4):
                    mm_gh(pb[m][:, BL:2 * BL], 8 + m, bank_first=True,
                          close=True)                            # hn (gh-only)

                # --- phase B: gx matmuls + sigmoids
                for m in range(4):
                    mm_gx(pa[m][:, 0:BL], m)                     # r close
                    rb = (btab_t[:, 16 + rz_ct * 8 + m:16 + rz_ct * 8 + m + 1]
                          if rz_ct is not None else 0.0)
                    nc.scalar.activation(rz[:, m, 0, :], pa[m][:, 0:BL], AF.Sigmoid,
                                         bias=rb)
                for m in range(4):
                    mm_gx(pa[m][:, BL:2 * BL], 4 + m)            # z close
                    zb = (btab_t[:, 16 + rz_ct * 8 + 4 + m:16 + rz_ct * 8 + 4 + m + 1]
                          if rz_ct is not None else 0.0)
                    nc.scalar.activation(rz[:, m, 1, :], pa[m][:, BL:2 * BL], AF.Sigmoid,
                                         bias=zb)
                for m in range(4):
                    mm_gx(pb[m][:, 0:BL], 8 + m)                 # xn

                # --- phase C: elementwise tail (per-chunk pipelined)
                for m in range(4):
                    # t = (hn + cn) * r
                    nc.vector.scalar_tensor_tensor(
                        tt[:, m, :], pb[m][:, BL:2 * BL],
                        btab_t[:, cn_ct * 4 + m:cn_ct * 4 + m + 1],
                        rz[:, m, 0, :], OP.add, OP.mult)
                    # v = (t + bn) + xn
                    bns = (btab_t[:, 40 + bn_ct * 4 + m:40 + bn_ct * 4 + m + 1]
                           if bn_ct is not None else 0.0)
                    nc.vector.scalar_tensor_tensor(
                        vv[:, m, :], tt[:, m, :], bns, pb[m][:, 0:BL],
                        OP.add, OP.add)
                for mm2 in range(2):
                    s = slice(2 * mm2, 2 * mm2 + 2)
                    nc.scalar.activation(nn[:, s, :], vv[:, s, :], AF.Tanh)
                for mm2 in range(2):
                    s = slice(2 * mm2, 2 * mm2 + 2)
                    # h' = n + z*(h - n)   (all-fp16 SBUF: DVE fast mode)
                    nc.vector.tensor_tensor(uu[:, s, :], h_ew[:, s, :], nn[:, s, :],
                                            OP.subtract)
                    nc.vector.tensor_tensor(pw[:, s, :], uu[:, s, :], rz[:, s, 1, :],
                                            OP.mult)
                    nc.vector.tensor_tensor(h_out[:, s, :], pw[:, s, :], nn[:, s, :],
                                            OP.add)

            for _rep in range(repeat):
                for i in range(2):
                    nc.vector.memzero(h0b[i][:])
                    nc.vector.memzero(h1b[i][:])

                # ---------------- encoder ----------------
                sc = None
                for t in range(lags):
                    if t % SRC_CHUNK == 0:
                        sc = sp.tile([F + 1, SRC_CHUNK, BL], F16, tag="src",
                                     name=f"src{t}")
                        nc.gpsimd.dma_start(sc[:], srcT_d[:, t:t + SRC_CHUNK, :])
                    j = t % SRC_CHUNK
                    p, q = t % 2, (t + 1) % 2
                    cell([sc[:, j, :]], [ew0a_t], wt["eu0"],
                         h0b[p], h0b[q], CN_EL0, None, None)
                    cell([h0b[q][:, k, :] for k in range(KC)], wt["ew1"], wt["eu1"],
                         h1b[p], h1b[q], CN_EL1, RZ_EL1, RZ_EL1)
                sc_last = sc

                # ---------------- decoder ----------------
                for d in range(horizons):
                    p, q = (lags + d) % 2, (lags + d + 1) % 2
                    if d == 0:
                        cell([sc_last[:, (lags - 1) % SRC_CHUNK, :]], [dw0a_t],
                             wt["du0"], h0b[p], h0b[q], CN_DL0, None, None)
                    else:
                        cell([h1b[p][:, k, :] for k in range(KC)], wt["wcomb"],
                             wt["du0"], h0b[p], h0b[q], CN_DL0, RZ_DL0R, RZ_DL0R)
                    cell([h0b[q][:, k, :] for k in range(KC)], wt["dw1"], wt["du1"],
                         h1b[p], h1b[q], CN_DL1, RZ_DL1, RZ_DL1)
                    # out1[d] = W1 . h1_new   (b1 added on host)
                    po = pp.tile([128, 512], F32, tag="pA0", name=f"po{d}")
                    for k in range(KC):
                        nc.tensor.matmul(po[0:1, 0:BL], w1t_t[:, k:k + 1],
                                         h1b[q][:, k, :], start=(k == 0),
                                         stop=(k == KC - 1))
                    osb = opool.tile([1, BL], F32, tag="o1", name=f"o{d}")
                    nc.scalar.copy(osb[:], po[0:1, 0:BL])
                    nc.sync.dma_start(out_d[d:d + 1, :], osb[:])

    nc.compile()
    return nc


def _host_prep(inputs):
    f32 = np.float32
    g = {k: np.asarray(v, dtype=f32) if np.asarray(v).dtype != np.int64 else v
         for k, v in inputs.items()}
    src = np.asarray(inputs["src"], f32)
    eW0, eU0, eb0, ec0 = g["eW0"], g["eU0"], g["eb0"], g["ec0"]
    eW1, eU1, eb1, ec1 = g["eW1"], g["eU1"], g["eb1"], g["ec1"]
    dW0, dU0, db0, dc0 = g["dW0"], g["dU0"], g["db0"], g["dc0"]
    dW1, dU1, db1, dc1 = g["dW1"], g["dU1"], g["db1"], g["dc1"]
    W1, b1, W4, b4 = g["W1"], g["b1"], g["W4"], g["b4"]

    def rzn_bias(b, c):
        # ones-row payload: r/z rows carry b+c, n row carries b only
        return np.concatenate([b[0:H] + c[0:H], b[H:2 * H] + c[H:2 * H], b[2 * H:]])

    Wcomb = (dW0 @ W4).astype(f32)                       # [1536, 512]
    dcomb = (db0 + dW0 @ b4).astype(f32)                 # [1536]

    npd = F16NP if MM_DT == "float16" else np.float32
    bf = lambda a: np.ascontiguousarray(a).astype(npd)
    shared = {
        "eu0": bf(eU0.T), "ew1": bf(eW1.T), "eu1": bf(eU1.T),
        "du0": bf(dU0.T), "dw1": bf(dW1.T), "du1": bf(dU1.T),
        "wcomb": bf(Wcomb.T),
        "ew0a": bf(np.concatenate([eW0.T, rzn_bias(eb0, ec0)[None, :]], 0)),
        "dw0a": bf(np.concatenate([dW0.T, rzn_bias(db0, dc0)[None, :]], 0)),
        "w1t": bf(W1[0].reshape(KC, 128).T),
    }

    btab = np.zeros((128, NBCOL), f32)
    # cn columns (h-side n bias, used in stt-t)
    for ci, c in enumerate((ec0, ec1, dc0, dc1)):
        cn = c[2 * H:]
        for m in range(KC):
            btab[:, ci * 4 + m] = cn[m * 128:(m + 1) * 128]
    # rz bias columns (b+c summed) and bn columns (x-side n bias)
    for ci, (b, c) in enumerate(((eb1, ec1), (dcomb, dc0), (db1, dc1))):
        bc = b + c
        for gate in range(2):
            for m in range(KC):
                btab[:, 16 + ci * 8 + gate * 4 + m] = \
                    bc[gate * H + m * 128:gate * H + (m + 1) * 128]
        bn = b[2 * H:]
        for m in range(KC):
            btab[:, 40 + ci * 4 + m] = bn[m * 128:(m + 1) * 128]
    shared["btab"] = btab

    in_maps = []
    for c in range(NCORES):
        s = src[c * BL:(c + 1) * BL]                     # [256, 64, 64]
        sT = np.ascontiguousarray(s.transpose(2, 1, 0))  # [64, 64, 256]
        sA = np.concatenate([sT, np.ones((1, LAGS, BL), f32)], 0)
        m = dict(shared)
        m["srcT"] = bf(sA)
        in_maps.append(m)
    return in_maps, float(b1[0])


class _Runner:
    """Build-once sharded PJRT runner (axon: 8 NeuronCores)."""

    def __init__(self, nc):
        import jax
        from jax.sharding import Mesh, PartitionSpec
        from jax.experimental.shard_map import shard_map
        from concourse import mybir
        from concourse.bass2jax import (_bass_exec_p, partition_id_tensor,
                                        install_neuronx_cc_hook)
        install_neuronx_cc_hook()
        self.jax = jax
        partition_name = nc.partition_id_tensor.name if nc.partition_id_tensor else None
        in_names, out_names, out_avals, zero_outs = [], [], [], []
        for alloc in nc.m.functions[0].allocations:
            if not isinstance(alloc, mybir.MemoryLocationSet):
                continue
            name = alloc.memorylocations[0].name
            if alloc.kind == "ExternalInput":
                if name != partition_name:
                    in_names.append(name)
            elif alloc.kind == "ExternalOutput":
                out_names.append(name)
                shape = tuple(alloc.tensor_shape)
                dtype = mybir.dt.np(alloc.dtype)
                out_avals.append(jax.core.ShapedArray(shape, dtype))
                zero_outs.append(np.zeros(shape, dtype))
        n_params = len(in_names)
        all_in = list(in_names) + list(out_names)
        if partition_name is not None:
            all_in.append(partition_name)
        self.in_names, self.out_names = in_names, out_names
        self.out_avals, self.zero_outs = out_avals, zero_outs

        def _body(*args):
            operands = list(args)
            if partition_name is not None:
                operands.append(partition_id_tensor())
            return tuple(_bass_exec_p.bind(
                *operands, out_avals=tuple(out_avals), in_names=tuple(all_in),
                out_names=tuple(out_names), lowering_input_output_aliases=(),
                sim_require_finite=True, sim_require_nnan=True, nc=nc))

        devices = jax.devices()[:NCORES]
        self.mesh = Mesh(np.asarray(devices), ("core",))
        in_specs = (PartitionSpec("core"),) * (n_params + len(out_names))
        out_specs = (PartitionSpec("core"),) * len(out_names)
        donate = tuple(range(n_params, n_params + len(out_names)))
        self.fn = jax.jit(
            shard_map(_body, mesh=self.mesh, in_specs=in_specs,
                      out_specs=out_specs, check_rep=False),
            donate_argnums=donate, keep_unused=True)
        self.sh = jax.sharding.NamedSharding(self.mesh, PartitionSpec("core"))

    def place(self, in_maps):
        n = NCORES
        self.placed = [
            self.jax.device_put(np.ascontiguousarray(
                np.concatenate([in_maps[c][nm] for c in range(n)], 0)), self.sh)
            for nm in self.in_names]

    def run(self):
        zeros = [self.jax.device_put(
            np.zeros((NCORES * z.shape[0], *z.shape[1:]), z.dtype), self.sh)
            for z in self.zero_outs]
        outs = self.fn(*self.placed, *zeros)
        self.jax.block_until_ready(outs)
        return outs

    def results(self, outs):
        return [
            {nm: np.asarray(outs[i]).reshape(NCORES, *self.out_avals[i].shape)[c]
             for i, nm in enumerate(self.out_names)}
            for c in range(NCORES)]


def get_runner(repeat=1):
    global _RUNNER
    key = ("r", repeat, MM_DT, EW16)
    if _RUNNER is None or _RUNNER[0] != key:
        nc = _build_nc(repeat=repeat)
        _RUNNER = (key, _Runner(nc))
    return _RUNNER[1]


def kernel(**inputs) -> np.ndarray:
    in_maps, b1 = _host_prep(inputs)
    r = get_runner()
    r.place(in_maps)
    res = r.results(r.run())
    out = np.empty((B, HORIZONS), np.float32)
    for c in range(NCORES):
        out[c * BL:(c + 1) * BL] = res[c]["out"].T + b1
    return out


# revision 11
# speedup vs baseline: 1.0126x; 1.0011x over previous
"""GRU Seq2Seq Trainium2 kernel (nn_GRU_Seq2Seq_83219286327778).

Strategy: data-parallel over batch (2048 -> 8 x 256), gate-major transposed
layout on-device ([hidden/gate dim on partitions, batch on free dim]) so the
recurrence needs no transposes.

v2: fp16 matmul operands (weights, hidden state, src) with fp32 PSUM
accumulation; biases applied via the ACT engine's per-partition bias operand
(sigmoid) and the DVE scalar_tensor_tensor scalar slots (tanh path) instead
of rank-1 matmuls; per-cell matmuls ordered gh-first so the recurrent-side
matmuls (which depend only on state from two cells back) fill the tensor
engine while the previous cell's elementwise tail completes; h-update runs
in fp16 SBUF (DVE 4x mode); fc4 feedback folded into the next step's gx via
Wcomb = dW0 @ W4; all weights resident in SBUF from the start.
"""
import sys
sys.path.insert(0, "/opt/trn_rl_repo")
import numpy as np

F16NP = np.float16

B, LAGS, HORIZONS, F, H = 2048, 64, 24, 64, 512
NCORES = 8
BL = B // NCORES           # 256 batch per core
G3 = 3 * H                 # 1536
KC = H // 128              # 4 k-chunks
SRC_CHUNK = 8              # timesteps per src DMA

# btab column layout (bias table, [128, 52] fp32):
#   cn   (stt-t scalar):  cols  0..15  = ctype {el0,el1,dl0,dl1} * 4 + m
#   rz   (ACT bias):      cols 16..39  = 16 + ctype {el1,dl0r,dl1}*8 + gate*4 + m
#   bn   (stt-v scalar):  cols 40..51  = 40 + ctype {el1,dl0r,dl1}*4 + m
CN_EL0, CN_EL1, CN_DL0, CN_DL1 = 0, 1, 2, 3
RZ_EL1, RZ_DL0R, RZ_DL1 = 0, 1, 2
NBCOL = 52

_RUNNER = None

# matmul operand dtype ('float16' | 'float32r') and fp16-elementwise toggle
MM_DT = "float16"
EW16 = True


def _build_nc(repeat=1, lags=LAGS, horizons=HORIZONS, mm_dt=None, ew16=None):
    import concourse.tile as tile
    from concourse import mybir, bacc

    mm_dt = MM_DT if mm_dt is None else mm_dt
    ew16 = EW16 if ew16 is None else ew16
    F32 = mybir.dt.float32
    F16 = getattr(mybir.dt, mm_dt)
    E16 = mybir.dt.float16 if ew16 else F32
    # DRAM dtype must match the numpy arrays the runner feeds (float32r
    # tiles are bit-identical to fp32, so DMA from an F32 dram tensor)
    FD = F32 if mm_dt == "float32r" else F16
    AF = mybir.ActivationFunctionType
    OP = mybir.AluOpType

    nc = bacc.Bacc("TRN2", target_bir_lowering=False)

    srcT_d = nc.dram_tensor("srcT", [F + 1, LAGS, BL], FD, kind="ExternalInput")
    wnames = ["eu0", "ew1", "eu1", "du0", "dw1", "du1", "wcomb"]
    w_d = {n: nc.dram_tensor(n, [H, G3], FD, kind="ExternalInput") for n in wnames}
    ew0a_d = nc.dram_tensor("ew0a", [F + 1, G3], FD, kind="ExternalInput")
    dw0a_d = nc.dram_tensor("dw0a", [F + 1, G3], FD, kind="ExternalInput")
    btab_d = nc.dram_tensor("btab", [128, NBCOL], F32, kind="ExternalInput")
    w1t_d = nc.dram_tensor("w1t", [128, KC], FD, kind="ExternalInput")
    out_d = nc.dram_tensor("out", [HORIZONS, BL], F32, kind="ExternalOutput")

    with tile.TileContext(nc) as tc:
        with tc.tile_pool(name="wp", bufs=1) as wp, \
             tc.tile_pool(name="sp", bufs=2) as sp, \
             tc.tile_pool(name="hp", bufs=1) as hp, \
             tc.tile_pool(name="gp", bufs=2) as gp, \
             tc.tile_pool(name="op_", bufs=2) as opool, \
             tc.tile_pool(name="pp", bufs=1, space="PSUM") as pp:

            # ---- persistent small tensors ----
            btab_t = wp.tile([128, NBCOL], F32, tag="btab", name="btab")
            nc.sync.dma_start(btab_t[:], btab_d[:])
            w1t_t = wp.tile([128, KC], F16, tag="w1t", name="w1t")
            nc.gpsimd.dma_start(w1t_t[:], w1t_d[:])
            ew0a_t = wp.tile([F + 1, G3], F16, tag="w0a", name="w0a")
            nc.gpsimd.dma_start(ew0a_t[:], ew0a_d[:])
            dw0a_t = wp.tile([F + 1, G3], F16, tag="dw0a", name="dw0a")
            nc.gpsimd.dma_start(dw0a_t[:], dw0a_d[:])

            _dmae = [nc.gpsimd, nc.sync]

            def load_u(name):
                ts_ = []
                for k in range(KC):
                    t = wp.tile([128, G3], F16, tag=f"{name}{k}", name=f"{name}{k}")
                    _dmae[k % 2].dma_start(t[:], w_d[name][k * 128:(k + 1) * 128, :])
                    ts_.append(t)
                return ts_

            wt = {n: load_u(n) for n in wnames}

            # hidden state ping-pong, fp16 (matmul rhs + h-update operand)
            h0b = [hp.tile([128, KC, BL], F16, tag=f"h0{i}", name=f"h0{i}") for i in range(2)]
            h1b = [hp.tile([128, KC, BL], F16, tag=f"h1{i}", name=f"h1{i}") for i in range(2)]

            def cell(gx_rhs, gx_lhs, gh_lhs, h_prev, h_out, cn_ct, rz_ct, bn_ct):
                """One GRU cell, gate-major layout, gh-first matmul order.

                gx_rhs/gx_lhs: k-chunk lists for the input projection
                gh_lhs: KC lhsT tiles for the recurrent projection
                cn_ct: ctype index into btab cn columns (always set)
                rz_ct: ctype index into btab rz columns, or None (biases folded
                       into gx via the ones-row of the [F+1] input)
                bn_ct: ctype index into btab bn columns, or None (ones-row)
                """
                pa = [pp.tile([128, 512], F32, tag=f"pA{m}", name=f"pA{m}") for m in range(4)]
                pb = [pp.tile([128, 512], F32, tag=f"pB{m}", name=f"pB{m}") for m in range(4)]
                rz = gp.tile([128, KC, 2, BL], E16, tag="rz", name="rz")
                tt = gp.tile([128, KC, BL], F32, tag="tt", name="tt")
                vv = gp.tile([128, KC, BL], F32, tag="vv", name="vv")
                nn = gp.tile([128, KC, BL], E16, tag="nn", name="nn")
                uu = gp.tile([128, KC, BL], E16, tag="uu", name="uu")
                pw = gp.tile([128, KC, BL], E16, tag="pw", name="pw")

                ngh = len(gh_lhs) if gh_lhs is not None else 0
                ngx = len(gx_lhs)
                # DVE reads of float32r tiles need an fp32 view
                h_ew = h_prev if mm_dt != "float32r" else h_prev.bitcast(F32)

                def mm_gh(out_ap, g, bank_first=False, close=False):
                    # start=True ONLY on the first matmul into the bank this
                    # cell: it clears has_written for the WHOLE bank. Later
                    # regions initialize via the per-element overwrite-on-
                    # unset-bit behavior (flags=0).
                    for k in range(ngh):
                        nc.tensor.matmul(out_ap, gh_lhs[k][:, g * 128:(g + 1) * 128],
                                         h_prev[:, k, :],
                                         start=(bank_first and k == 0),
                                         stop=(close and k == ngh - 1),
                                         skip_group_check=True)

                def mm_gx(out_ap, g):
                    for i, (lhs, rhs) in enumerate(zip(gx_lhs, gx_rhs, strict=True)):
                        nc.tensor.matmul(out_ap, lhs[:, g * 128:(g + 1) * 128], rhs,
                                         start=False, stop=(i == ngx - 1),
                                         skip_group_check=True)

                # --- phase A: gh matmuls (depend only on 2-cells-back state)
                if ngh:
                    for m in range(4):
                        mm_gh(pa[m][:, 0:BL], m, bank_first=True)    # r
                    for m in range(4):
                        mm_gh(pa[m][:, BL:2 * BL], 4 + m)            # z
                    for m in range(4):
                        mm_gh(pb[m][:, BL:2 * BL], 8 + m, bank_first=True,
                              close=True)                            # hn (gh-only)

                # --- phase B: gx matmuls + sigmoids
                for m in range(4):
                    if not ngh and gx_lhs:
                        # zero-state cell: this is the bank's first matmul
                        nc.tensor.matmul(pa[m][:, 0:BL],
                                         gx_lhs[0][:, m * 128:(m + 1) * 128],
                                         gx_rhs[0], start=True, stop=(ngx == 1),
                                         skip_group_check=True)
                        for i in range(1, ngx):
                            nc.tensor.matmul(pa[m][:, 0:BL],
                                             gx_lhs[i][:, m * 128:(m + 1) * 128],
                                             gx_rhs[i], start=False,
                                             stop=(i == ngx - 1),
                                             skip_group_check=True)
                    else:
                        mm_gx(pa[m][:, 0:BL], m)                 # r close
                    rb = (btab_t[:, 16 + rz_ct * 8 + m:16 + rz_ct * 8 + m + 1]
                          if rz_ct is not None else 0.0)
                    nc.scalar.activation(rz[:, m, 0, :], pa[m][:, 0:BL], AF.Sigmoid,
                                         bias=rb)
                for m in range(4):
                    mm_gx(pa[m][:, BL:2 * BL], 4 + m)            # z close
                    zb = (btab_t[:, 16 + rz_ct * 8 + 4 + m:16 + rz_ct * 8 + 4 + m + 1]
                          if rz_ct is not None else 0.0)
                    nc.scalar.activation(rz[:, m, 1, :], pa[m][:, BL:2 * BL], AF.Sigmoid,
                                         bias=zb)
                for m in range(4):
                    if not ngh:
                        nc.tensor.matmul(pb[m][:, 0:BL],
                                         gx_lhs[0][:, (8 + m) * 128:(9 + m) * 128],
                                         gx_rhs[0], start=True, stop=(ngx == 1),
                                         skip_group_check=True)
                        for i in range(1, ngx):
                            nc.tensor.matmul(pb[m][:, 0:BL],
                                             gx_lhs[i][:, (8 + m) * 128:(9 + m) * 128],
                                             gx_rhs[i], start=False,
                                             stop=(i == ngx - 1),
                                             skip_group_check=True)
                    else:
                        mm_gx(pb[m][:, 0:BL], 8 + m)             # xn

                # --- phase C: elementwise tail (per-chunk pipelined)
                for m in range(4):
                    # t = (hn + cn) * r     (hn = 0 at the zero-state step)
                    if ngh:
                        nc.vector.scalar_tensor_tensor(
                            tt[:, m, :], pb[m][:, BL:2 * BL],
                            btab_t[:, cn_ct * 4 + m:cn_ct * 4 + m + 1],
                            rz[:, m, 0, :], OP.add, OP.mult)
                    else:
                        nc.vector.tensor_scalar(
                            tt[:, m, :], rz[:, m, 0, :],
                            btab_t[:, cn_ct * 4 + m:cn_ct * 4 + m + 1],
                            None, OP.mult)
                    # v = (t + bn) + xn
                    bns = (btab_t[:, 40 + bn_ct * 4 + m:40 + bn_ct * 4 + m + 1]
                           if bn_ct is not None else 0.0)
                    nc.vector.scalar_tensor_tensor(
                        vv[:, m, :], tt[:, m, :], bns, pb[m][:, 0:BL],
                        OP.add, OP.add)
                for mm2 in range(2):
                    s = slice(2 * mm2, 2 * mm2 + 2)
                    nc.scalar.activation(nn[:, s, :], vv[:, s, :], AF.Tanh)
                for mm2 in range(2):
                    s = slice(2 * mm2, 2 * mm2 + 2)
                    # h' = n + z*(h - n)   (all-fp16 SBUF: DVE fast mode)
                    nc.vector.tensor_tensor(uu[:, s, :], h_ew[:, s, :], nn[:, s, :],
                                            OP.subtract)
                    nc.vector.tensor_tensor(pw[:, s, :], uu[:, s, :], rz[:, s, 1, :],
                                            OP.mult)
                    nc.vector.tensor_tensor(h_out[:, s, :], pw[:, s, :], nn[:, s, :],
                                            OP.add)

            for _rep in range(repeat):
                for i in range(2):
                    nc.vector.memzero(h0b[i][:])
                    nc.vector.memzero(h1b[i][:])

                # ---------------- encoder ----------------
                sc = None
                for t in range(lags):
                    if t % SRC_CHUNK == 0:
                        sc = sp.tile([F + 1, SRC_CHUNK, BL], F16, tag="src",
                                     name=f"src{t}")
                        nc.gpsimd.dma_start(sc[:], srcT_d[:, t:t + SRC_CHUNK, :])
                    j = t % SRC_CHUNK
                    p, q = t % 2, (t + 1) % 2
                    cell([sc[:, j, :]], [ew0a_t],
                         wt["eu0"] if t else None,
                         h0b[p], h0b[q], CN_EL0, None, None)
                    cell([h0b[q][:, k, :] for k in range(KC)], wt["ew1"],
                         wt["eu1"] if t else None,
                         h1b[p], h1b[q], CN_EL1, RZ_EL1, RZ_EL1)
                sc_last = sc

                # ---------------- decoder ----------------
                for d in range(horizons):
                    p, q = (lags + d) % 2, (lags + d + 1) % 2
                    if d == 0:
                        cell([sc_last[:, (lags - 1) % SRC_CHUNK, :]], [dw0a_t],
                             wt["du0"], h0b[p], h0b[q], CN_DL0, None, None)
                    else:
                        cell([h1b[p][:, k, :] for k in range(KC)], wt["wcomb"],
                             wt["du0"], h0b[p], h0b[q], CN_DL0, RZ_DL0R, RZ_DL0R)
                    cell([h0b[q][:, k, :] for k in range(KC)], wt["dw1"], wt["du1"],
                         h1b[p], h1b[q], CN_DL1, RZ_DL1, RZ_DL1)
                    # out1[d] = W1 . h1_new   (b1 added on host)
                    po = pp.tile([128, 512], F32, tag="pA0", name=f"po{d}")
                    for k in range(KC):
                        nc.tensor.matmul(po[0:1, 0:BL], w1t_t[:, k:k + 1],
                                         h1b[q][:, k, :], start=(k == 0),
                                         stop=(k == KC - 1))
                    osb = opool.tile([1, BL], F32, tag="o1", name=f"o{d}")
                    nc.scalar.copy(osb[:], po[0:1, 0:BL])
                    nc.sync.dma_start(out_d[d:d + 1, :], osb[:])

    nc.compile()
    return nc


def _host_prep(inputs):
    f32 = np.float32
    g = {k: np.asarray(v, dtype=f32) if np.asarray(v).dtype != np.int64 else v
         for k, v in inputs.items()}
    src = np.asarray(inputs["src"], f32)
    eW0, eU0, eb0, ec0 = g["eW0"], g["eU0"], g["eb0"], g["ec0"]
    eW1, eU1, eb1, ec1 = g["eW1"], g["eU1"], g["eb1"], g["ec1"]
    dW0, dU0, db0, dc0 = g["dW0"], g["dU0"], g["db0"], g["dc0"]
    dW1, dU1, db1, dc1 = g["dW1"], g["dU1"], g["db1"], g["dc1"]
    W1, b1, W4, b4 = g["W1"], g["b1"], g["W4"], g["b4"]

    def rzn_bias(b, c):
        # ones-row payload: r/z rows carry b+c, n row carries b only
        return np.concatenate([b[0:H] + c[0:H], b[H:2 * H] + c[H:2 * H], b[2 * H:]])

    Wcomb = (dW0 @ W4).astype(f32)                       # [1536, 512]
    dcomb = (db0 + dW0 @ b4).astype(f32)                 # [1536]

    npd = F16NP if MM_DT == "float16" else np.float32
    bf = lambda a: np.ascontiguousarray(a).astype(npd)
    shared = {
        "eu0": bf(eU0.T), "ew1": bf(eW1.T), "eu1": bf(eU1.T),
        "du0": bf(dU0.T), "dw1": bf(dW1.T), "du1": bf(dU1.T),
        "wcomb": bf(Wcomb.T),
        "ew0a": bf(np.concatenate([eW0.T, rzn_bias(eb0, ec0)[None, :]], 0)),
        "dw0a": bf(np.concatenate([dW0.T, rzn_bias(db0, dc0)[None, :]], 0)),
        "w1t": bf(W1[0].reshape(KC, 128).T),
    }

    btab = np.zeros((128, NBCOL), f32)
    # cn columns (h-side n bias, used in stt-t)
    for ci, c in enumerate((ec0, ec1, dc0, dc1)):
        cn = c[2 * H:]
        for m in range(KC):
            btab[:, ci * 4 + m] = cn[m * 128:(m + 1) * 128]
    # rz bias columns (b+c summed) and bn columns (x-side n bias)
    for ci, (b, c) in enumerate(((eb1, ec1), (dcomb, dc0), (db1, dc1))):
        bc = b + c
        for gate in range(2):
            for m in range(KC):
                btab[:, 16 + ci * 8 + gate * 4 + m] = \
                    bc[gate * H + m * 128:gate * H + (m + 1) * 128]
        bn = b[2 * H:]
        for m in range(KC):
            btab[:, 40 + ci * 4 + m] = bn[m * 128:(m + 1) * 128]
    shared["btab"] = btab

    in_maps = []
    for c in range(NCORES):
        s = src[c * BL:(c + 1) * BL]                     # [256, 64, 64]
        sT = np.ascontiguousarray(s.transpose(2, 1, 0))  # [64, 64, 256]
        sA = np.concatenate([sT, np.ones((1, LAGS, BL), f32)], 0)
        m = dict(shared)
        m["srcT"] = bf(sA)
        in_maps.append(m)
    return in_maps, float(b1[0])


class _Runner:
    """Build-once sharded PJRT runner (axon: 8 NeuronCores)."""

    def __init__(self, nc):
        import jax
        from jax.sharding import Mesh, PartitionSpec
        from jax.experimental.shard_map import shard_map
        from concourse import mybir
        from concourse.bass2jax import (_bass_exec_p, partition_id_tensor,
                                        install_neuronx_cc_hook)
        install_neuronx_cc_hook()
        self.jax = jax
        partition_name = nc.partition_id_tensor.name if nc.partition_id_tensor else None
        in_names, out_names, out_avals, zero_outs = [], [], [], []
        for alloc in nc.m.functions[0].allocations:
            if not isinstance(alloc, mybir.MemoryLocationSet):
                continue
            name = alloc.memorylocations[0].name
            if alloc.kind == "ExternalInput":
                if name != partition_name:
                    in_names.append(name)
            elif alloc.kind == "ExternalOutput":
                out_names.append(name)
                shape = tuple(alloc.tensor_shape)
                dtype = mybir.dt.np(alloc.dtype)
                out_avals.append(jax.core.ShapedArray(shape, dtype))
                zero_outs.append(np.zeros(shape, dtype))
        n_params = len(in_names)
        all_in = list(in_names) + list(out_names)
        if partition_name is not None:
            all_in.append(partition_name)
        self.in_names, self.out_names = in_names, out_names
        self.out_avals, self.zero_outs = out_avals, zero_outs

        def _body(*args):
            operands = list(args)
            if partition_name is not None:
                operands.append(partition_id_tensor())
            return tuple(_bass_exec_p.bind(
                *operands, out_avals=tuple(out_avals), in_names=tuple(all_in),
                out_names=tuple(out_names), lowering_input_output_aliases=(),
                sim_require_finite=True, sim_require_nnan=True, nc=nc))

        devices = jax.devices()[:NCORES]
        self.mesh = Mesh(np.asarray(devices), ("core",))
        in_specs = (PartitionSpec("core"),) * (n_params + len(out_names))
        out_specs = (PartitionSpec("core"),) * len(out_names)
        donate = tuple(range(n_params, n_params + len(out_names)))
        self.fn = jax.jit(
            shard_map(_body, mesh=self.mesh, in_specs=in_specs,
                      out_specs=out_specs, check_rep=False),
            donate_argnums=donate, keep_unused=True)
        self.sh = jax.sharding.NamedSharding(self.mesh, PartitionSpec("core"))

    def place(self, in_maps):
        n = NCORES
        self.placed = [
            self.jax.device_put(np.ascontiguousarray(
                np.concatenate([in_maps[c][nm] for c in range(n)], 0)), self.sh)
            for nm in self.in_names]

    def run(self):
        zeros = [self.jax.device_put(
            np.zeros((NCORES * z.shape[0], *z.shape[1:]), z.dtype), self.sh)
            for z in self.zero_outs]
        outs = self.fn(*self.placed, *zeros)
        self.jax.block_until_ready(outs)
        return outs

    def results(self, outs):
        return [
            {nm: np.asarray(outs[i]).reshape(NCORES, *self.out_avals[i].shape)[c]
             for i, nm in enumerate(self.out_names)}
            for c in range(NCORES)]


def get_runner(repeat=1):
    global _RUNNER
    key = ("r", repeat, MM_DT, EW16)
    if _RUNNER is None or _RUNNER[0] != key:
        nc = _build_nc(repeat=repeat)
        _RUNNER = (key, _Runner(nc))
    return _RUNNER[1]


def kernel(**inputs) -> np.ndarray:
    in_maps, b1 = _host_prep(inputs)
    r = get_runner()
    r.place(in_maps)
    res = r.results(r.run())
    out = np.empty((B, HORIZONS), np.float32)
    for c in range(NCORES):
        out[c * BL:(c + 1) * BL] = res[c]["out"].T + b1
    return out
